# revision 25
# baseline (speedup 1.0000x reference)
"""LocalAttention3D Trainium2 kernel (Gram-shared, jk-sharded, fully
pipelined).

Problem: x [B=2, C=1, D=96, H=64, W=64], per-head scalar-affine q/k/v
projections (NH=4), scores = einsum('bdjk,bdlm->bjklm', q, k)/sqrt(32),
softmax over the last W axis (windows of 64), out = attn @ v, summed over
heads.

Key algebra: q/k/v are scalar affine in x, so with G = x^T x (Gram,
head-independent) and u = colsum(x):
  S_h[jk,lm]/sqrt(32) = a_h*G + b_h*u[jk] + g_h*u[lm] + c_h
The b_h*u[jk] and c_h terms are constant along both softmax-window axes
(l,m), i.e. they scale numerator and denominator of the softmax equally
-> dropped exactly.  A global shift M=15 also cancels and keeps 1/Z away
from f32 denormals.  Per head (one ACT op per tile):
  E_h^T[lm, jk] = exp(a_h * G^T + g_h*u[lm] - M)
  (scale = a_h via [128,1] AP, bias = per-partition AP, host-fed)

Sharding: core = (b, jk-quarter of 1024).  All 4 heads share one
G^T[lm-tile, jk-slice] matmul (float32r: full fp32 at 1 col/cycle for
>=256 moving cols).  w_v[h] is folded into 1/Z, so out accumulates over
lm-tiles AND heads in one PSUM bank; softmax rows sum to 1 so b_v
contributes exactly 64*sum_h(b_v[h]), added at evac.  Output needs no
host reduction, just concatenation of jk slices.

The softmax windows l=2t,2t+1 live entirely inside lm-tile t, so the
denominators are PER-TILE local: a block-ones matmul drops each tile's
window sums for all 4 heads into one packed PSUM tile per TG-tile group
(row = h*2*TG + tt*2 + g), giving a single flat software pipeline with
no phase barrier:
  per tile t: G_t (PE f32r) -> 4x exp (ACT) -> z matmul (PE)
  per TG-tile group: 1/z (DVE) * w_v -> bf16 zj;
  per (t, h): broadcast zj row pair across 64 partitions each (one DMA,
    alternating SP/Pool trigger); P' = E * zb (DVE bf16 2x);
    av += XT_t^T @ P' (PE).
Emission interleaves group g's normalize/apply units between group
g+1's tile steps so the per-engine program order never serializes the
pipeline (PE would otherwise run all of C(g) before starting A(g+1)).
"""

import math
import sys

sys.path.insert(0, "/opt/trn_rl_repo")

import numpy as np
import ml_dtypes

import bass_rust
import concourse.bass as bass
import concourse.tile as tile
from concourse import mybir
from concourse.bass_utils import run_bass_kernel_spmd

BF16 = ml_dtypes.bfloat16

B, D, HW = 2, 96, 64 * 64
NH = 4
NCORES = 8
JKC = 1024            # jk columns per core (HW / 4 quarters)
NT = HW // 128        # 32 lm-tiles of 128 partitions (2 softmax windows each)
TG = 4                # lm-tiles per z-group
NG = NT // TG         # groups
SCALE = 1.0 / math.sqrt(32.0)
MSHIFT = 15.0         # global exponent shift (cancels in softmax)


def _split_excess_waits(nc, max_waits=1):
    """This container's walrus rejects instructions with >1 semaphore wait
    ("Too many sync wait commands"). Move extra waits onto no-op carriers
    inserted just before the instruction on the same engine."""
    ctr = 0
    for f in nc.m.functions:
        for blk in f.blocks:
            insts = blk.instructions
            out = []
            changed = False
            for ins in insts:
                try:
                    si = ins.sync_info
                except Exception:
                    si = None
                if si is not None and len(si.on_wait) > max_waits:
                    waits = list(si.on_wait)
                    for w in waits[:-max_waits]:
                        ctr += 1
                        nop = mybir.InstNoOp(
                            name=f"wsplit-{ctr}-{ins.name}", ins=[], outs=[])
                        nop.engine = ins.engine
                        nop.sync_info = bass_rust.SyncInfo(
                            on_wait=[w], on_update=[])
                        nc.register_instruction(nop, overwrite=True)
                        out.append(nop)
                        changed = True
                    ins.sync_info = bass_rust.SyncInfo(
                        on_wait=waits[-max_waits:], on_update=list(si.on_update))
                out.append(ins)
            if changed:
                blk.instructions = out


def _build_program():
    f32 = mybir.dt.float32
    f32r = mybir.dt.float32r
    bf16 = mybir.dt.bfloat16

    nc = bass.Bass("TRN2", target_bir_lowering=False, debug=False,
                   num_devices=1)
    x_d = nc.dram_tensor("x", [D, HW], f32r, kind="ExternalInput").ap()
    xj_d = nc.dram_tensor("xj", [D, JKC], f32r, kind="ExternalInput").ap()
    xt_d = nc.dram_tensor("xt", [128, NT * D], bf16,
                          kind="ExternalInput").ap()
    u2g_d = nc.dram_tensor("u2g", [128, NH * NT], f32,
                           kind="ExternalInput").ap()
    wqk_d = nc.dram_tensor("wqk", [128, NH], f32, kind="ExternalInput").ap()
    wvz_d = nc.dram_tensor("wvz", [128, 1], f32, kind="ExternalInput").ap()
    bvc_d = nc.dram_tensor("bvc", [D, 1], f32, kind="ExternalInput").ap()
    boz_d = nc.dram_tensor("boz", [128, NH * TG * 2 * TG * NH], bf16,
                           kind="ExternalInput").ap()
    out_d = nc.dram_tensor("out", [D, JKC], f32, kind="ExternalOutput").ap()

    mult, add = mybir.AluOpType.mult, mybir.AluOpType.add
    exp = mybir.ActivationFunctionType.Exp

    with tile.TileContext(nc) as tc:
        with (
            tc.tile_pool(name="cn", bufs=1) as cn,
            tc.tile_pool(name="ew", bufs=44) as ew,
            tc.tile_pool(name="zr", bufs=2) as zrp,
            tc.tile_pool(name="zf", bufs=2) as zfp,
            tc.tile_pool(name="zb", bufs=18) as zbp,
            tc.tile_pool(name="pt", bufs=6) as ptp,
            tc.tile_pool(name="ob", bufs=1) as obp,
            tc.tile_pool(name="ps_s", bufs=2, space="PSUM") as ps_s,
            tc.tile_pool(name="ps_z", bufs=2, space="PSUM") as ps_z,
            tc.tile_pool(name="ps_av", bufs=1, space="PSUM") as ps_av,
        ):
            X = cn.tile([D, HW], f32r, tag="X")
            XJ = cn.tile([D, JKC], f32r, tag="XJ")
            XT = cn.tile([128, NT * D], bf16, tag="XT")
            U2G = cn.tile([128, NH * NT], f32, tag="U2G")
            WQK = cn.tile([128, NH], f32, tag="WQK")
            WVZ = cn.tile([128, 1], f32, tag="WVZ")
            BVC = cn.tile([D, 1], f32, tag="BVC")
            BOZ = cn.tile([128, NH * TG * 2 * TG * NH], bf16, tag="BOZ")
            # Staged loads: the first G tile needs only X[:, :128] + XJ +
            # exp scale/bias; everything later arrives behind the pipeline.
            nc.sync.dma_start(X[:, :512], x_d[:, :512])
            nc.sync.dma_start(XJ[:], xj_d[:])
            nc.sync.dma_start(U2G[:], u2g_d[:])
            nc.sync.dma_start(WQK[:], wqk_d[:])
            nc.gpsimd.dma_start(X[:, 512:HW // 2], x_d[:, 512:HW // 2])
            nc.scalar.dma_start(X[:, HW // 2:], x_d[:, HW // 2:])
            nc.gpsimd.dma_start(BOZ[:], boz_d[:])
            nc.gpsimd.dma_start(XT[:], xt_d[:])
            nc.sync.dma_start(WVZ[:], wvz_d[:])
            nc.sync.dma_start(BVC[:], bvc_d[:])

            av = ps_av.tile([D, JKC], f32, tag="av")

            NR = 2 * TG               # z rows per head (uniform layout)
            # Last group split in two so the pipeline drain is short.
            sizes = [TG] * (NT // TG - 1) + [TG // 2, TG - TG // 2]
            starts = [sum(sizes[:i]) for i in range(len(sizes))]
            group_of = {}
            for gi, (st0, sz) in enumerate(zip(starts, sizes)):
                for tt in range(sz):
                    group_of[st0 + tt] = (gi, tt)
            ets, zgs, zjs = {}, {}, {}
            n_mm2 = [0]
            total_mm2 = NH * NT

            def emit_g_exp(t):
                g8, tt = group_of[t]
                st = ps_s.tile([128, JKC], f32, tag="st",
                               name=f"st_{g8}_{tt}")
                xs = X[:, t * 128:(t + 1) * 128]
                for hh in range(2):
                    nc.tensor.matmul(
                        st[:, hh * 512:(hh + 1) * 512],
                        xs,
                        XJ[:, hh * 512:(hh + 1) * 512],
                        start=True, stop=True)
                for h in range(NH):
                    et = ew.tile([128, JKC], bf16, tag="et",
                                 name=f"et_{g8}_{tt}_{h}")
                    nc.scalar.activation(
                        et[:], st[:], exp,
                        bias=U2G[:, h * NT + t:h * NT + t + 1],
                        scale=WQK[:, h:h + 1])
                    ets[(g8, h, tt)] = et

            def emit_z(t):
                g8, tt = group_of[t]
                if tt == 0:
                    zgs[g8] = [
                        ps_z.tile([NR * NH, 512], f32, tag="zg",
                                  name=f"zg_{g8}_{i}") for i in range(2)]
                zw = NR * NH
                for h in range(NH):
                    et = ets[(g8, h, tt)]
                    bz = BOZ[:, (h * TG + tt) * zw:(h * TG + tt + 1) * zw]
                    for hh in range(2):
                        nc.tensor.matmul(
                            zgs[g8][hh][:, :],
                            bz,
                            et[:, hh * 512:(hh + 1) * 512],
                            start=(tt == 0 and h == 0),
                            stop=(tt == sizes[g8] - 1 and h == NH - 1))

            def emit_b(g8):
                zj = zfp.tile([NR * NH, JKC], bf16, tag="zj",
                              name=f"zj_{g8}")
                for hh in range(2):
                    zr = zrp.tile([NR * NH, 512], f32, tag="zr",
                                  name=f"zr_{g8}_{hh}")
                    nc.vector.reciprocal(zr[:], zgs[g8][hh][:])
                    nc.vector.tensor_scalar(zj[:, hh * 512:(hh + 1) * 512],
                                            zr[:], WVZ[:NR * NH, 0:1],
                                            None, mult)
                zjs[g8] = zj

            zbs = {}

            def emit_zb(g8, h, tt):
                zb = zbp.tile([128, JKC], bf16, tag="zb",
                              name=f"zb_{g8}_{tt}_{h}")
                r0 = h * NR + 2 * tt
                src = zjs[g8][r0:r0 + 2, :].unsqueeze(1).broadcast_to(
                    (2, 64, JKC))
                eng = nc.sync if (h + tt) % 2 else nc.gpsimd
                eng.dma_start(zb[:], src)
                zbs[(g8, h, tt)] = zb

            def emit_c(g8, h, tt):
                t = starts[g8] + tt
                ptt = ptp.tile([128, JKC], bf16, tag="pt",
                               name=f"pt_{g8}_{tt}_{h}")
                nc.vector.tensor_mul(ptt[:], ets.pop((g8, h, tt))[:],
                                     zbs.pop((g8, h, tt))[:])
                vt = XT[:, t * D:(t + 1) * D]
                k = n_mm2[0]
                n_mm2[0] += 1
                for hh in range(2):
                    nc.tensor.matmul(
                        av[:, hh * 512:(hh + 1) * 512],
                        vt,
                        ptt[:, hh * 512:(hh + 1) * 512],
                        start=(k == 0),
                        stop=(k == total_mm2 - 1))

            # Flat stage-lagged pipeline over global tile steps s:
            #   G+exp at step s; z-matmuls lag 1 step (so PE's G for step
            #   s+1 never queues behind a z that waits on step s's exps);
            #   group g's denominators complete at step g*TG+TG, its
            #   normalize + 16 broadcast DMAs fire then, and its 16
            #   apply-units spread NH-per-step over the following steps.
            b_at = {starts[g] + sizes[g]: g for g in range(len(sizes))}
            pending = []
            for s in range(NT + 8):
                if s < NT:
                    emit_g_exp(s)
                if 1 <= s <= NT:
                    emit_z(s - 1)
                if s in b_at:
                    gb = b_at[s]
                    emit_b(gb)
                    for h in range(NH):
                        for tt2 in range(sizes[gb]):
                            emit_zb(gb, h, tt2)
                            pending.append((gb, h, tt2))
                take = len(pending) if s >= NT else min(NH, len(pending))
                for gc, h, tt2 in pending[:take]:
                    emit_c(gc, h, tt2)
                del pending[:take]
                if s >= NT and not pending and s + 1 > NT:
                    break
            ob = obp.tile([D, JKC], f32, tag="ob")
            nc.vector.tensor_scalar(ob[:], av[:], BVC[:, 0:1], None, add)
            nc.sync.dma_start(out_d[:], ob[:])

    _split_excess_waits(nc)
    return nc


_NC = None


def _get_program():
    global _NC
    if _NC is None:
        _NC = _build_program()
    return _NC


def _make_in_maps(x, wq, bq, wk, bk, wv, bv):
    x = np.asarray(x, dtype=np.float32)
    x2 = x.reshape(B, D, HW)
    wq, bq, wk, bk, wv, bv = [
        np.asarray(a, dtype=np.float32) for a in (wq, bq, wk, bk, wv, bv)]

    # z-matmul stationary: block (h, tt) maps partition-group g to packed
    # row h*2*TG + tt*2 + g
    zw = 2 * TG * NH
    boz = np.zeros((128, NH * TG * zw), dtype=BF16)
    for h in range(NH):
        for tt in range(TG):
            for g in range(2):
                boz[g * 64:(g + 1) * 64,
                    (h * TG + tt) * zw + h * 2 * TG + tt * 2 + g] = BF16(1.0)

    alpha = wq * wk * SCALE                    # [NH] exp scale per head
    gamma = bq * wk * SCALE                    # [NH] coeff of u[lm]

    wqk = np.broadcast_to(alpha[None, :], (128, NH)).copy()
    wvz = np.ones((128, 1), dtype=np.float32)
    for h in range(NH):
        wvz[h * 2 * TG:(h + 1) * 2 * TG, 0] = wv[h]
    assert 2 * TG * NH <= 128
    bvc = np.full((D, 1), 64.0 * bv.sum(), dtype=np.float32)

    per_b = {}
    for b in range(B):
        xb = x2[b]
        xt = np.ascontiguousarray(
            xb.reshape(D, NT, 128).transpose(2, 1, 0).reshape(
                128, NT * D)).astype(BF16)
        u2 = xb.sum(axis=0)                    # [HW]
        u2t = u2.reshape(NT, 128).T            # [128, NT]
        u2g = np.zeros((128, NH * NT), dtype=np.float32)
        for h in range(NH):
            u2g[:, h * NT:(h + 1) * NT] = gamma[h] * u2t - MSHIFT
        per_b[b] = (np.ascontiguousarray(xb), xt, u2g)

    in_maps = []
    for c in range(NCORES):
        b, q = divmod(c, NCORES // B)
        xb, xt, u2g = per_b[b]
        in_maps.append({
            "x": xb,
            "xj": np.ascontiguousarray(xb[:, q * JKC:(q + 1) * JKC]),
            "xt": xt,
            "u2g": u2g,
            "wqk": wqk,
            "wvz": wvz,
            "bvc": bvc,
            "boz": boz,
        })
    return in_maps


def kernel(x, wq, bq, wk, bk, wv, bv):
    nc = _get_program()
    in_maps = _make_in_maps(x, wq, bq, wk, bk, wv, bv)
    res = run_bass_kernel_spmd(nc, in_maps, core_ids=list(range(NCORES)))
    out = np.zeros((B, 1, D, 64, 64), dtype=np.float32)
    for c in range(NCORES):
        b, q = divmod(c, NCORES // B)
        out[b, 0].reshape(D, HW)[:, q * JKC:(q + 1) * JKC] = (
            res.results[c]["out"])
    return out


# revision 35
# speedup vs baseline: 51.9149x; 51.9149x over previous
"""LocalAttention3D Trainium2 kernel (Gram-shared, jk-sharded, fully
pipelined).

Problem: x [B=2, C=1, D=96, H=64, W=64], per-head scalar-affine q/k/v
projections (NH=4), scores = einsum('bdjk,bdlm->bjklm', q, k)/sqrt(32),
softmax over the last W axis (windows of 64), out = attn @ v, summed over
heads.

Key algebra: q/k/v are scalar affine in x, so with G = x^T x (Gram,
head-independent) and u = colsum(x):
  S_h[jk,lm]/sqrt(32) = a_h*G + b_h*u[jk] + g_h*u[lm] + c_h
The b_h*u[jk] and c_h terms are constant along both softmax-window axes
(l,m), i.e. they scale numerator and denominator of the softmax equally
-> dropped exactly.  A global shift M=15 also cancels and keeps 1/Z away
from f32 denormals.  Per head (one ACT op per tile):
  E_h^T[lm, jk] = exp(a_h * G^T + g_h*u[lm] - M)
  (scale = a_h via [128,1] AP, bias = per-partition AP, host-fed)

Sharding: core = (b, jk-quarter of 1024).  All 4 heads share one
G^T[lm-tile, jk-slice] matmul (float32r: full fp32 at 1 col/cycle for
>=256 moving cols).  w_v[h] is folded into 1/Z, so out accumulates over
lm-tiles AND heads in one PSUM bank; softmax rows sum to 1 so b_v
contributes exactly 64*sum_h(b_v[h]), added at evac.  Output needs no
host reduction, just concatenation of jk slices.

The softmax windows l=2t,2t+1 live entirely inside lm-tile t, so the
denominators are PER-TILE local: a block-ones matmul drops each tile's
window sums for all 4 heads into one packed PSUM tile per TG-tile group
(row = h*2*TG + tt*2 + g), giving a single flat software pipeline with
no phase barrier:
  per tile t: G_t (PE f32r) -> 4x exp (ACT) -> z matmul (PE)
  per TG-tile group: 1/z (DVE) * w_v -> bf16 zj;
  per (t, h): broadcast zj row pair across 64 partitions each (one DMA,
    alternating SP/Pool trigger); P' = E * zb (DVE bf16 2x);
    av += XT_t^T @ P' (PE).
Emission interleaves group g's normalize/apply units between group
g+1's tile steps so the per-engine program order never serializes the
pipeline (PE would otherwise run all of C(g) before starting A(g+1)).
"""

import math
import sys

sys.path.insert(0, "/opt/trn_rl_repo")

import numpy as np
import ml_dtypes

import bass_rust
import concourse.bass as bass
import concourse.tile as tile
from concourse import mybir
from concourse.bass_utils import run_bass_kernel_spmd

BF16 = ml_dtypes.bfloat16

B, D, HW = 2, 96, 64 * 64
NH = 4
NCORES = 8
JKC = 1024            # jk columns per core (HW / 4 quarters)
NT = HW // 128        # 32 lm-tiles of 128 partitions (2 softmax windows each)
TG = 4                # lm-tiles per z-group
NG = NT // TG         # groups
SCALE = 1.0 / math.sqrt(32.0)
MSHIFT = 15.0         # global exponent shift (cancels in softmax)


def _split_excess_waits(nc, max_waits=1):
    """This container's walrus rejects instructions with >1 semaphore wait
    ("Too many sync wait commands"). Move extra waits onto no-op carriers
    inserted just before the instruction on the same engine."""
    ctr = 0
    for f in nc.m.functions:
        for blk in f.blocks:
            insts = blk.instructions
            out = []
            changed = False
            for ins in insts:
                try:
                    si = ins.sync_info
                except Exception:
                    si = None
                if si is not None and len(si.on_wait) > max_waits:
                    waits = list(si.on_wait)
                    for w in waits[:-max_waits]:
                        ctr += 1
                        nop = mybir.InstNoOp(
                            name=f"wsplit-{ctr}-{ins.name}", ins=[], outs=[])
                        nop.engine = ins.engine
                        nop.sync_info = bass_rust.SyncInfo(
                            on_wait=[w], on_update=[])
                        nc.register_instruction(nop, overwrite=True)
                        out.append(nop)
                        changed = True
                    ins.sync_info = bass_rust.SyncInfo(
                        on_wait=waits[-max_waits:], on_update=list(si.on_update))
                out.append(ins)
            if changed:
                blk.instructions = out


def _build_program():
    f32 = mybir.dt.float32
    f32r = mybir.dt.float32r
    bf16 = mybir.dt.bfloat16

    nc = bass.Bass("TRN2", target_bir_lowering=False, debug=False,
                   num_devices=1)
    x_d = nc.dram_tensor("x", [D, HW], f32r, kind="ExternalInput").ap()
    xj_d = nc.dram_tensor("xj", [D, JKC], f32r, kind="ExternalInput").ap()
    xt_d = nc.dram_tensor("xt", [128, NT * D], bf16,
                          kind="ExternalInput").ap()
    u2g_d = nc.dram_tensor("u2g", [128, NH * NT], f32,
                           kind="ExternalInput").ap()
    wqk_d = nc.dram_tensor("wqk", [128, NH], f32, kind="ExternalInput").ap()
    wvz_d = nc.dram_tensor("wvz", [128, 1], f32, kind="ExternalInput").ap()
    bvc_d = nc.dram_tensor("bvc", [D, 1], f32, kind="ExternalInput").ap()
    boz_d = nc.dram_tensor("boz", [128, NH * TG * 2 * TG * NH], bf16,
                           kind="ExternalInput").ap()
    out_d = nc.dram_tensor("out", [D, JKC], f32, kind="ExternalOutput").ap()

    mult, add = mybir.AluOpType.mult, mybir.AluOpType.add
    exp = mybir.ActivationFunctionType.Exp

    with tile.TileContext(nc) as tc:
        with (
            tc.tile_pool(name="cn", bufs=1) as cn,
            tc.tile_pool(name="ew", bufs=44) as ew,
            tc.tile_pool(name="zr", bufs=2) as zrp,
            tc.tile_pool(name="zf", bufs=2) as zfp,
            tc.tile_pool(name="zb", bufs=18) as zbp,
            tc.tile_pool(name="pt", bufs=6) as ptp,
            tc.tile_pool(name="ob", bufs=1) as obp,
            tc.tile_pool(name="ps_s", bufs=2, space="PSUM") as ps_s,
            tc.tile_pool(name="ps_z", bufs=1, space="PSUM") as ps_z,
            tc.tile_pool(name="ps_av", bufs=1, space="PSUM") as ps_av,
        ):
            X = cn.tile([D, HW], f32r, tag="X")
            XJ = cn.tile([D, JKC], f32r, tag="XJ")
            XT = cn.tile([128, NT * D], bf16, tag="XT")
            U2G = cn.tile([128, NH * NT], f32, tag="U2G")
            WQK = cn.tile([128, NH], f32, tag="WQK")
            WVZ = cn.tile([128, 1], f32, tag="WVZ")
            BVC = cn.tile([D, 1], f32, tag="BVC")
            BOZ = cn.tile([128, NH * TG * 2 * TG * NH], bf16, tag="BOZ")
            # Staged loads: the first G tile needs only X[:, :128] + XJ +
            # exp scale/bias; everything later arrives behind the pipeline.
            nc.sync.dma_start(X[:, :512], x_d[:, :512])
            nc.sync.dma_start(XJ[:], xj_d[:])
            nc.sync.dma_start(U2G[:], u2g_d[:])
            nc.sync.dma_start(WQK[:], wqk_d[:])
            nc.gpsimd.dma_start(X[:, 512:HW // 2], x_d[:, 512:HW // 2])
            nc.gpsimd.dma_start(X[:, HW // 2:], x_d[:, HW // 2:])
            nc.gpsimd.dma_start(BOZ[:], boz_d[:])
            nc.gpsimd.dma_start(XT[:], xt_d[:])
            nc.sync.dma_start(WVZ[:], wvz_d[:])
            nc.sync.dma_start(BVC[:], bvc_d[:])

            av = ps_av.tile([D, JKC], f32, tag="av")

            NR = 2 * TG               # z rows per head (uniform layout)
            # Last group split in two so the pipeline drain is short.
            sizes = [TG] * (NT // TG - 1) + [TG - 1, 1]
            starts = [sum(sizes[:i]) for i in range(len(sizes))]
            group_of = {}
            for gi, (st0, sz) in enumerate(zip(starts, sizes)):
                for tt in range(sz):
                    group_of[st0 + tt] = (gi, tt)
            ets, zgs, zjs = {}, {}, {}
            n_mm2 = [0]
            total_mm2 = NH * NT

            def emit_g_exp(t):
                g8, tt = group_of[t]
                st = ps_s.tile([128, JKC], f32, tag="st",
                               name=f"st_{g8}_{tt}")
                xs = X[:, t * 128:(t + 1) * 128]
                for hh in range(2):
                    nc.tensor.matmul(
                        st[:, hh * 512:(hh + 1) * 512],
                        xs,
                        XJ[:, hh * 512:(hh + 1) * 512],
                        start=True, stop=True)
                for h in range(NH):
                    et = ew.tile([128, JKC], bf16, tag="et",
                                 name=f"et_{g8}_{tt}_{h}")
                    nc.scalar.activation(
                        et[:], st[:], exp,
                        bias=U2G[:, h * NT + t:h * NT + t + 1],
                        scale=WQK[:, h:h + 1])
                    ets[(g8, h, tt)] = et

            def emit_z(t):
                g8, tt = group_of[t]
                sup, lane = divmod(g8, 2)
                if lane == 0 and tt == 0:
                    zgs[sup] = [
                        ps_z.tile([128, 512], f32, tag=f"zs{i}",
                                  name=f"zs_{sup}_{i}") for i in range(2)]
                zw = NR * NH
                r = slice(lane * 64, lane * 64 + zw)
                for h in range(NH):
                    et = ets[(g8, h, tt)]
                    bz = BOZ[:, (h * TG + tt) * zw:(h * TG + tt + 1) * zw]
                    for hh in range(2):
                        nc.tensor.matmul(
                            zgs[sup][hh][r, :],
                            bz,
                            et[:, hh * 512:(hh + 1) * 512],
                            start=(tt == 0 and h == 0),
                            stop=(tt == sizes[g8] - 1 and h == NH - 1))

            def emit_b(g8):
                sup, lane = divmod(g8, 2)
                zw = NR * NH
                r = slice(lane * 64, lane * 64 + zw)
                zj = zfp.tile([NR * NH, JKC], bf16, tag="zj",
                              name=f"zj_{g8}")
                for hh in range(2):
                    zr = zrp.tile([NR * NH, 512], f32, tag="zr",
                                  name=f"zr_{g8}_{hh}")
                    nc.vector.reciprocal(zr[:], zgs[sup][hh][r, :])
                    nc.vector.tensor_scalar(zj[:, hh * 512:(hh + 1) * 512],
                                            zr[:], WVZ[:NR * NH, 0:1],
                                            None, mult)
                zjs[g8] = zj

            zbs = {}

            def emit_zb(g8, h, tt):
                zb = zbp.tile([128, JKC], bf16, tag="zb",
                              name=f"zb_{g8}_{tt}_{h}")
                r0 = h * NR + 2 * tt
                src = zjs[g8][r0:r0 + 2, :].unsqueeze(1).broadcast_to(
                    (2, 64, JKC))
                eng = nc.sync if (h + tt) % 2 else nc.gpsimd
                eng.dma_start(zb[:], src)
                zbs[(g8, h, tt)] = zb

            def emit_c(g8, h, tt):
                t = starts[g8] + tt
                ptt = ptp.tile([128, JKC], bf16, tag="pt",
                               name=f"pt_{g8}_{tt}_{h}")
                nc.vector.tensor_mul(ptt[:], ets.pop((g8, h, tt))[:],
                                     zbs.pop((g8, h, tt))[:])
                vt = XT[:, t * D:(t + 1) * D]
                k = n_mm2[0]
                n_mm2[0] += 1
                for hh in range(2):
                    nc.tensor.matmul(
                        av[:, hh * 512:(hh + 1) * 512],
                        vt,
                        ptt[:, hh * 512:(hh + 1) * 512],
                        start=(k == 0),
                        stop=(k == total_mm2 - 1))

            # Flat stage-lagged pipeline over global tile steps s:
            #   G+exp at step s; z-matmuls lag 1 step (so PE's G for step
            #   s+1 never queues behind a z that waits on step s's exps);
            #   group g's denominators complete at step g*TG+TG, its
            #   normalize + 16 broadcast DMAs fire then, and its 16
            #   apply-units spread NH-per-step over the following steps.
            b_at = {starts[g] + sizes[g]: g for g in range(len(sizes))}
            pending = []
            for s in range(NT + 8):
                if s < NT:
                    emit_g_exp(s)
                if 1 <= s <= NT:
                    emit_z(s - 1)
                if s in b_at:
                    gb = b_at[s]
                    emit_b(gb)
                    for h in range(NH):
                        for tt2 in range(sizes[gb]):
                            emit_zb(gb, h, tt2)
                            pending.append((gb, h, tt2))
                take = len(pending) if s > NT else min(NH, len(pending))
                for gc, h, tt2 in pending[:take]:
                    emit_c(gc, h, tt2)
                del pending[:take]
            # Split evac so the first half's store overlaps the second
            # half's bias-add.
            ob = obp.tile([D, JKC], f32, tag="ob")
            for hh in range(2):
                sl = slice(hh * 512, (hh + 1) * 512)
                nc.vector.tensor_scalar(ob[:, sl], av[:, sl], BVC[:, 0:1],
                                        None, add)
                eng = nc.sync if hh == 0 else nc.gpsimd
                eng.dma_start(out_d[:, sl], ob[:, sl])

    _split_excess_waits(nc)
    return nc


_NC = None


def _get_program():
    global _NC
    if _NC is None:
        _NC = _build_program()
    return _NC


def _make_in_maps(x, wq, bq, wk, bk, wv, bv):
    x = np.asarray(x, dtype=np.float32)
    x2 = x.reshape(B, D, HW)
    wq, bq, wk, bk, wv, bv = [
        np.asarray(a, dtype=np.float32) for a in (wq, bq, wk, bk, wv, bv)]

    # z-matmul stationary: block (h, tt) maps partition-group g to packed
    # row h*2*TG + tt*2 + g
    zw = 2 * TG * NH
    boz = np.zeros((128, NH * TG * zw), dtype=BF16)
    for h in range(NH):
        for tt in range(TG):
            for g in range(2):
                boz[g * 64:(g + 1) * 64,
                    (h * TG + tt) * zw + h * 2 * TG + tt * 2 + g] = BF16(1.0)

    alpha = wq * wk * SCALE                    # [NH] exp scale per head
    gamma = bq * wk * SCALE                    # [NH] coeff of u[lm]

    wqk = np.broadcast_to(alpha[None, :], (128, NH)).copy()
    wvz = np.ones((128, 1), dtype=np.float32)
    for h in range(NH):
        wvz[h * 2 * TG:(h + 1) * 2 * TG, 0] = wv[h]
    assert 2 * TG * NH <= 128
    bvc = np.full((D, 1), 64.0 * bv.sum(), dtype=np.float32)

    per_b = {}
    for b in range(B):
        xb = x2[b]
        xt = np.ascontiguousarray(
            xb.reshape(D, NT, 128).transpose(2, 1, 0).reshape(
                128, NT * D)).astype(BF16)
        u2 = xb.sum(axis=0)                    # [HW]
        u2t = u2.reshape(NT, 128).T            # [128, NT]
        u2g = np.zeros((128, NH * NT), dtype=np.float32)
        for h in range(NH):
            u2g[:, h * NT:(h + 1) * NT] = gamma[h] * u2t - MSHIFT
        per_b[b] = (np.ascontiguousarray(xb), xt, u2g)

    in_maps = []
    for c in range(NCORES):
        b, q = divmod(c, NCORES // B)
        xb, xt, u2g = per_b[b]
        in_maps.append({
            "x": xb,
            "xj": np.ascontiguousarray(xb[:, q * JKC:(q + 1) * JKC]),
            "xt": xt,
            "u2g": u2g,
            "wqk": wqk,
            "wvz": wvz,
            "bvc": bvc,
            "boz": boz,
        })
    return in_maps


def kernel(x, wq, bq, wk, bk, wv, bv):
    nc = _get_program()
    in_maps = _make_in_maps(x, wq, bq, wk, bk, wv, bv)
    res = run_bass_kernel_spmd(nc, in_maps, core_ids=list(range(NCORES)))
    out = np.zeros((B, 1, D, 64, 64), dtype=np.float32)
    for c in range(NCORES):
        b, q = divmod(c, NCORES // B)
        out[b, 0].reshape(D, HW)[:, q * JKC:(q + 1) * JKC] = (
            res.results[c]["out"])
    return out
